# revision 19
# baseline (speedup 1.0000x reference)
"""Trainium2 Bass kernel for nn_MessagePassing (vertical message passing).

Computation (per batch element b):
    y[0] = x[0]
    y[i] = x[i] + relu(conv1d_same(y[i-1], W))   for i = 1..H-1
with x (H, W, C) = (128, 256, 128) fp32, W (K, Cin, Cout) = (9, 128, 128).

Sharding: batch B=8 across the 8 NeuronCores (data parallel, no
communication). Each core runs the sequential H recurrence for one batch
element.

Per-core layout: the recurrent state is kept *transposed* in SBUF as
yT (C=128 partitions, W+8 columns with 4 zero pad columns each side).
One step is 9 accumulating matmuls psum[co, w] += W[k].T @ yT[:, k:k+256]
(float32r: fp32 data truncated to FP22 internally -> full PE rate at
free-dim 256), then a single fused DVE op
    yT_new = max(psum_conv, 0) + psum_xT
where psum_xT holds the transpose of the incoming x row (done on the PE in
exact fp32 transpose mode). Output rows are transposed back (PE) and
DMA'd out contiguously.
"""

import numpy as np

B, H, W_DIM, C, K = 8, 128, 256, 128, 9
PAD = 4
WBUF = W_DIM + 2 * PAD  # 264
P = 128

_NC_CACHE = {}


def _build_nc():
    import concourse.tile as tile
    from concourse import bacc, mybir
    from concourse.masks import make_identity

    f32 = mybir.dt.float32
    f32r = mybir.dt.float32r

    nc = bacc.Bacc("TRN2", target_bir_lowering=False, debug=False, num_devices=B)
    x_d = nc.dram_tensor("x", [H, W_DIM, C], f32, kind="ExternalInput").ap()
    w_d = nc.dram_tensor("w", [K, C, C], f32, kind="ExternalInput").ap()
    o_d = nc.dram_tensor("out", [H, W_DIM, C], f32, kind="ExternalOutput").ap()

    with tile.TileContext(nc) as tc:
        with (
            tc.tile_pool(name="const", bufs=1) as const_pool,
            tc.tile_pool(name="xin", bufs=6) as xin_pool,
            tc.tile_pool(name="state", bufs=1) as state_pool,
            tc.tile_pool(name="xT", bufs=3) as xT_pool,
            tc.tile_pool(name="stage", bufs=4) as stage_pool,
            tc.tile_pool(name="pconv", bufs=2, space="PSUM") as pconv_pool,
            tc.tile_pool(name="px", bufs=2, space="PSUM") as px_pool,
            tc.tile_pool(name="pout", bufs=2, space="PSUM") as pout_pool,
        ):
            ident = const_pool.tile([P, P], f32, name="ident")
            make_identity(nc, ident[:])
            # walrus requires every producer feeding an fp32r matmul to round
            # its output to fp32r, so materialize fp32r copies via DVE
            ident_r = const_pool.tile([P, P], f32r, name="ident_r")
            nc.vector.tensor_copy(ident_r[:], ident[:])

            # weights -> SBUF as (ci partitions, K, co), rounded to fp32r
            wsb_raw = const_pool.tile([P, K, C], f32, name="wsb_raw")
            nc.sync.dma_start(wsb_raw[:], w_d.rearrange("k ci co -> ci k co"))
            wsb = const_pool.tile([P, K, C], f32r, name="wsb")
            nc.vector.tensor_copy(wsb[:], wsb_raw[:])

            # double-buffered transposed state (fp32r so the conv matmuls can
            # read it directly); zero the whole buffer once via a DVE copy
            # (memset can't emit fp32r ISA, a rounding tensor_copy can)
            zbuf = const_pool.tile([P, WBUF], f32, name="zbuf")
            nc.vector.memset(zbuf[:], 0.0)
            yT = []
            for j in range(2):
                t = state_pool.tile([P, WBUF], f32r, name=f"yT{j}", tag=f"yT{j}")
                nc.vector.tensor_copy(t[:], zbuf[:])
                yT.append(t)

            # row 0 of the output is x[0] verbatim
            nc.sync.dma_start(o_d[0], x_d[0])

            x_tiles = {}

            def load_x(i):
                if i >= H:
                    return
                t = xin_pool.tile([P, 2, C], f32, tag="xt")
                nc.sync.dma_start(t[:], x_d[i].rearrange("(t w) c -> w t c", t=2))
                x_tiles[i] = t

            PREFETCH = 4
            for i in range(PREFETCH):
                load_x(i)

            def transpose_pair(dst_psum, src_a, src_b, ident=ident):
                # two (p, 128) srcs -> (p, 256) dst, each half transposed;
                # both halves share one PSUM accumulation group (single
                # bank / zero region).
                nc.tensor.matmul(
                    dst_psum[:, 0:C], src_a, ident[:],
                    is_transpose=True, start=True, stop=False,
                )
                nc.tensor.matmul(
                    dst_psum[:, C : 2 * C], src_b, ident[:],
                    is_transpose=True, start=False, stop=True,
                )

            # y_0 = x_0: transpose into PSUM, copy into yT[0]
            px0 = px_pool.tile([P, W_DIM], f32, tag="px")
            transpose_pair(px0, x_tiles[0][:, 0, :], x_tiles[0][:, 1, :])
            nc.vector.tensor_copy(yT[0][:, PAD : PAD + W_DIM], px0[:])

            # xT for step 1 (transpose via PE, then park in SBUF: the DVE can
            # read only one PSUM operand per instruction, so the residual
            # operand of the fused relu+add must come from SBUF)
            def make_xT(i):
                px = px_pool.tile([P, W_DIM], f32, tag="px")
                transpose_pair(px, x_tiles[i][:, 0, :], x_tiles[i][:, 1, :])
                xs = xT_pool.tile([P, W_DIM], f32, tag="xT")
                nc.vector.tensor_copy(xs[:], px[:])
                return xs

            xT_for = {1: make_xT(1)}

            po_prev = None
            for i in range(1, H):
                a, b = (i - 1) % 2, i % 2

                # 9 accumulating conv matmuls (float32r, N=256)
                pc = pconv_pool.tile([P, W_DIM], f32, tag="pconv")
                for k in range(K):
                    nc.tensor.matmul(
                        pc[:],
                        wsb[:, k, :],
                        yT[a][:, k : k + W_DIM],
                        start=(k == 0),
                        stop=(k == K - 1),
                    )

                # transpose row i-1 back to natural layout (row 0 already
                # written via the direct DRAM->DRAM copy)
                # fused relu + residual: yT[b] = max(conv, 0) + xT_i
                # (pc in PSUM, xT in SBUF — only one PSUM operand allowed)
                nc.vector.scalar_tensor_tensor(
                    yT[b][:, PAD : PAD + W_DIM],
                    pc[:],
                    0.0,
                    xT_for.pop(i)[:],
                    op0=mybir.AluOpType.max,
                    op1=mybir.AluOpType.add,
                )

                # xT for step i+1 (keeps PE busy while DVE does relu+add)
                if i + 1 < H:
                    xT_for[i + 1] = make_xT(i + 1)

                # transpose row i-1 back to natural layout and write it out
                # (row 0 already written via the direct DRAM->DRAM copy);
                # stage copy goes to the scalar engine to keep DVE free
                if i >= 2:
                    po = pout_pool.tile([P, W_DIM], f32r, tag="po")
                    transpose_pair(
                        po,
                        yT[a][:, PAD : PAD + C],
                        yT[a][:, PAD + C : PAD + W_DIM],
                        ident=ident_r,
                    )
                    st = stage_pool.tile([P, 2, C], f32, tag="stage")
                    nc.scalar.copy(st[:].rearrange("p t c -> p (t c)"), po[:])
                    nc.sync.dma_start(
                        o_d[i - 1].rearrange("(t w) c -> w t c", t=2), st[:]
                    )

                load_x(i - 1 + PREFETCH)
                x_tiles.pop(i - 1, None)

            # epilogue: final row H-1
            yl = yT[(H - 1) % 2]
            po = pout_pool.tile([P, W_DIM], f32r, tag="po")
            transpose_pair(
                po, yl[:, PAD : PAD + C], yl[:, PAD + C : PAD + W_DIM], ident=ident_r
            )
            st = stage_pool.tile([P, 2, C], f32, tag="stage")
            nc.scalar.copy(st[:].rearrange("p t c -> p (t c)"), po[:])
            nc.sync.dma_start(o_d[H - 1].rearrange("(t w) c -> w t c", t=2), st[:])

    nc.compile()
    return nc


def _get_nc():
    if "nc" not in _NC_CACHE:
        _NC_CACHE["nc"] = _build_nc()
    return _NC_CACHE["nc"]


def kernel(x, W):
    from concourse.bass_utils import run_bass_kernel_spmd

    x = np.asarray(x, dtype=np.float32)
    W = np.asarray(W, dtype=np.float32)
    assert x.shape == (B, H, W_DIM, C), x.shape
    assert W.shape == (K, C, C), W.shape

    nc = _get_nc()
    in_maps = [{"x": np.ascontiguousarray(x[b]), "w": W} for b in range(B)]
    res = run_bass_kernel_spmd(nc, in_maps, core_ids=list(range(B)))
    return np.stack([res.results[b]["out"] for b in range(B)], axis=0)


# revision 22
# speedup vs baseline: 312.2075x; 312.2075x over previous
"""Trainium2 Bass kernel for nn_MessagePassing (vertical message passing).

Computation (per batch element b):
    y[0] = x[0]
    y[i] = x[i] + relu(conv1d_same(y[i-1], W))   for i = 1..H-1
with x (H, W, C) = (128, 256, 128) fp32, W (K, Cin, Cout) = (9, 128, 128).

Sharding: batch B=8 across the 8 NeuronCores (data parallel, no
communication). Each core runs the sequential H recurrence for one batch
element.

Per-core layout: the recurrent state is kept *transposed* in SBUF as
yT (C=128 partitions, W+8 columns with 4 zero pad columns each side).
One step is 9 accumulating matmuls psum[co, w] += W[k].T @ yT[:, k:k+256]
(float32r: fp32 data truncated to FP22 internally -> full PE rate at
free-dim 256), then a single fused DVE op
    yT_new = max(psum_conv, 0) + xT
where xT is the transpose of the incoming x row (done on the PE in exact
fp32 transpose mode, parked in SBUF). Output rows are transposed back (PE)
and DMA'd out contiguously.
"""

import numpy as np

B, H, W_DIM, C, K = 8, 128, 256, 128, 9
PAD = 4
WBUF = W_DIM + 2 * PAD  # 264
P = 128

_NC_CACHE = {}


def _emit_body(nc, mybir, f32, f32r, x_d, o_d, pools, ident, ident_r, wsb, zbuf):
    (xin_pool, state_pool, xT_pool, stage_pool, pconv_pool, px_pool,
     pout_pool) = pools

    # double-buffered transposed state (fp32r so the conv matmuls can read
    # it directly); zeroed via a DVE copy (memset can't emit fp32r ISA, a
    # rounding tensor_copy can)
    yT = []
    for j in range(2):
        t = state_pool.tile([P, WBUF], f32r, tag=f"yT{j}", name=f"yT{j}")
        nc.vector.tensor_copy(t[:], zbuf[:])
        yT.append(t)

    # row 0 of the output is x[0] verbatim
    nc.sync.dma_start(o_d[0], x_d[0])

    x_tiles = {}

    def load_x(i):
        if i >= H:
            return
        t = xin_pool.tile([P, 2, C], f32, tag="xt", name=f"xt{i}")
        nc.sync.dma_start(t[:], x_d[i].rearrange("(t w) c -> w t c", t=2))
        x_tiles[i] = t

    PREFETCH = 4
    for i in range(PREFETCH):
        load_x(i)

    def transpose_pair(dst_psum, src_a, src_b, tident):
        # two (p, 128) srcs -> (p, 256) dst, each half transposed; both
        # halves share one PSUM accumulation group (single bank/zero region)
        nc.tensor.matmul(
            dst_psum[:, 0:C], src_a, tident[:],
            is_transpose=True, start=True, stop=False,
        )
        nc.tensor.matmul(
            dst_psum[:, C : 2 * C], src_b, tident[:],
            is_transpose=True, start=False, stop=True,
        )

    # y_0 = x_0: transpose into PSUM, copy into yT[0]
    px0 = px_pool.tile([P, W_DIM], f32, tag="px", name="px0")
    transpose_pair(px0, x_tiles[0][:, 0, :], x_tiles[0][:, 1, :], ident)
    nc.vector.tensor_copy(yT[0][:, PAD : PAD + W_DIM], px0[:])

    # xT rows: transpose via PE, then park in SBUF (the DVE can read only
    # one PSUM operand per instruction, so the residual operand of the
    # fused relu+add must come from SBUF)
    def make_xT(i):
        px = px_pool.tile([P, W_DIM], f32, tag="px", name=f"px{i}")
        transpose_pair(px, x_tiles[i][:, 0, :], x_tiles[i][:, 1, :], ident)
        xs = xT_pool.tile([P, W_DIM], f32, tag="xT", name=f"xT{i}")
        nc.vector.tensor_copy(xs[:], px[:])
        return xs

    xT_for = {1: make_xT(1)}

    for i in range(1, H):
        a, b = (i - 1) % 2, i % 2

        # 9 accumulating conv matmuls (float32r, N=256)
        pc = pconv_pool.tile([P, W_DIM], f32, tag="pconv", name=f"pc{i}")
        for k in range(K):
            nc.tensor.matmul(
                pc[:],
                wsb[:, k, :],
                yT[a][:, k : k + W_DIM],
                start=(k == 0),
                stop=(k == K - 1),
            )

        # fused relu + residual: yT[b] = max(conv, 0) + xT_i
        nc.vector.scalar_tensor_tensor(
            yT[b][:, PAD : PAD + W_DIM],
            pc[:],
            0.0,
            xT_for.pop(i)[:],
            op0=mybir.AluOpType.max,
            op1=mybir.AluOpType.add,
        )

        # xT for step i+1 (keeps PE busy while DVE does relu+add)
        if i + 1 < H:
            xT_for[i + 1] = make_xT(i + 1)

        # transpose row i-1 back to natural layout and write it out (row 0
        # already written via the direct DRAM->DRAM copy); the staging copy
        # goes to the scalar engine to keep DVE off the critical path
        if i >= 2:
            po = pout_pool.tile([P, W_DIM], f32r, tag="po", name=f"po{i}")
            transpose_pair(
                po,
                yT[a][:, PAD : PAD + C],
                yT[a][:, PAD + C : PAD + W_DIM],
                ident_r,
            )
            st = stage_pool.tile([P, 2, C], f32, tag="stage", name=f"st{i}")
            nc.scalar.copy(st[:].rearrange("p t c -> p (t c)"), po[:])
            nc.sync.dma_start(o_d[i - 1].rearrange("(t w) c -> w t c", t=2), st[:])

        load_x(i - 1 + PREFETCH)
        x_tiles.pop(i - 1, None)

    # epilogue: final row H-1
    yl = yT[(H - 1) % 2]
    po = pout_pool.tile([P, W_DIM], f32r, tag="po", name="po_last")
    transpose_pair(
        po, yl[:, PAD : PAD + C], yl[:, PAD + C : PAD + W_DIM], ident_r
    )
    st = stage_pool.tile([P, 2, C], f32, tag="stage", name="st_last")
    nc.scalar.copy(st[:].rearrange("p t c -> p (t c)"), po[:])
    nc.sync.dma_start(o_d[H - 1].rearrange("(t w) c -> w t c", t=2), st[:])


def _build_nc(reps=1):
    """Build the kernel module. reps>1 wraps the whole computation in a
    hardware loop that repeats it (identical work each trip) — used only to
    measure device execution time above the dispatch-noise floor."""
    import contextlib

    import concourse.tile as tile
    from concourse import bacc, mybir
    from concourse.masks import make_identity

    f32 = mybir.dt.float32
    f32r = mybir.dt.float32r

    nc = bacc.Bacc("TRN2", target_bir_lowering=False, debug=False, num_devices=B)
    x_d = nc.dram_tensor("x", [H, W_DIM, C], f32, kind="ExternalInput").ap()
    w_d = nc.dram_tensor("w", [K, C, C], f32, kind="ExternalInput").ap()
    o_d = nc.dram_tensor("out", [H, W_DIM, C], f32, kind="ExternalOutput").ap()

    with tile.TileContext(nc) as tc:
        with (
            tc.tile_pool(name="xin", bufs=6) as xin_pool,
            tc.tile_pool(name="state", bufs=1) as state_pool,
            tc.tile_pool(name="xT", bufs=3) as xT_pool,
            tc.tile_pool(name="stage", bufs=4) as stage_pool,
            tc.tile_pool(name="const", bufs=1) as const_pool,
            tc.tile_pool(name="pconv", bufs=2, space="PSUM") as pconv_pool,
            tc.tile_pool(name="px", bufs=2, space="PSUM") as px_pool,
            tc.tile_pool(name="pout", bufs=2, space="PSUM") as pout_pool,
        ):
            ident = const_pool.tile([P, P], f32, name="ident")
            make_identity(nc, ident[:])
            # walrus requires every producer feeding an fp32r matmul to round
            # its output to fp32r, so materialize fp32r copies via DVE
            ident_r = const_pool.tile([P, P], f32r, name="ident_r")
            nc.vector.tensor_copy(ident_r[:], ident[:])

            # weights -> SBUF as (ci partitions, K, co), rounded to fp32r
            wsb_raw = const_pool.tile([P, K, C], f32, name="wsb_raw")
            nc.sync.dma_start(wsb_raw[:], w_d.rearrange("k ci co -> ci k co"))
            wsb = const_pool.tile([P, K, C], f32r, name="wsb")
            nc.vector.tensor_copy(wsb[:], wsb_raw[:])

            zbuf = const_pool.tile([P, WBUF], f32, name="zbuf")
            nc.vector.memset(zbuf[:], 0.0)

            pools = (xin_pool, state_pool, xT_pool, stage_pool, pconv_pool,
                     px_pool, pout_pool)
            rep_ctx = tc.For_i(0, reps, 1) if reps > 1 else contextlib.nullcontext()
            with rep_ctx:
                _emit_body(nc, mybir, f32, f32r, x_d, o_d, pools, ident,
                           ident_r, wsb, zbuf)

    nc.compile()
    return nc


def _get_nc():
    if "nc" not in _NC_CACHE:
        _NC_CACHE["nc"] = _build_nc()
    return _NC_CACHE["nc"]


def kernel(x, W):
    from concourse.bass_utils import run_bass_kernel_spmd

    x = np.asarray(x, dtype=np.float32)
    W = np.asarray(W, dtype=np.float32)
    assert x.shape == (B, H, W_DIM, C), x.shape
    assert W.shape == (K, C, C), W.shape

    nc = _get_nc()
    in_maps = [{"x": np.ascontiguousarray(x[b]), "w": W} for b in range(B)]
    res = run_bass_kernel_spmd(nc, in_maps, core_ids=list(range(B)))
    return np.stack([res.results[b]["out"] for b in range(B)], axis=0)
